# revision 8
# baseline (speedup 1.0000x reference)
"""DeformableConv Trainium2 Bass kernel (transpose-free main loop).

B=8, Cin=128, Cout=256, H=W=64, K=3. Data-parallel over batch: core b
processes sample b. Per-core pipeline:

  1. offset conv (PE, 9 shifted-AP matmuls, M=18) -> offsets [18, 4096]
  2. PE-transpose offsets -> [128 pix, (pt,t,2)] layout; sampling
     coords, bilinear corner weights, pair-gather indices on DVE (fp32).
  3. Corner weights land in two bf16 planes (even = w00/w10 and
     odd = w01/w11, both [pix, (pt,t,row)]), are PE-transposed to
     (pt,t,row)-major, interleaved into (even,odd) pairs by strided ACT
     copies, collapsed per pixel-tile to one partition, and finally
     DMA partition-broadcast so each tile gets a [128chan, t, row,
     pix, 2] weight tile (identical rows on all channel partitions).
  4. ap_gather (gpsimd, d=1 float32 = one bf16 horizontal PAIR per
     index) fetches top-row and bottom-row corner pairs from the
     dual-phase padded image, one call per (row, pixel tile).
  5. DVE: two 2x-mode bf16 multiplies (pairs x weight-pairs) and one
     add implement the full bilinear weighting in channel-major
     layout; no per-tile transposes anywhere.
  6. PE conv: out[o,p] accumulated over 9 taps x 2 horizontal phases
     in PSUM (the phase pair-sum rides on the matmul accumulation);
     ACT drains to an 8-tile stage, DMA stores.
"""

import sys, os
sys.path.insert(0, "/opt/trn_rl_repo")

import numpy as np
import ml_dtypes

import concourse.bass as bass
import concourse.tile as tile
from concourse import bacc, mybir
from concourse.bass_utils import run_bass_kernel_spmd
from contextlib import ExitStack

F32 = mybir.dt.float32
BF16 = mybir.dt.bfloat16
I16 = mybir.dt.int16
I32 = mybir.dt.int32
ALU = mybir.AluOpType

B, CIN, COUT, H, W = 8, 128, 256, 64, 64
K2 = 9
HW = H * W                 # 4096
PADW = 70
FLAT = PADW * PADW         # 4900
SRCLEN = 9808              # [0:4900] padded img, [4900:9799] img shifted 1
NPT = 32                   # pixel tiles of 128
OG = 8                     # pixel tiles per output DMA group

_cache = {}


def _build_program():
    nc = bacc.Bacc("TRN2", target_bir_lowering=False, debug=False, num_devices=B)

    x_ext = nc.declare_dram_parameter("x", [CIN, HW], F32, isOutput=False)
    woff_ext = nc.declare_dram_parameter("woff", [CIN, K2, 18], BF16, isOutput=False)
    wr_ext = nc.declare_dram_parameter("wr", [CIN, K2, 2, 128], BF16, isOutput=False)
    idb_ext = nc.declare_dram_parameter("idb", [128, 128], BF16, isOutput=False)
    ybk_ext = nc.declare_dram_parameter("ybk", [128, 288], F32, isOutput=False)
    xbk_ext = nc.declare_dram_parameter("xbk", [128, 288], F32, isOutput=False)
    out_ext = nc.declare_dram_parameter("out", [2, 128, HW], F32, isOutput=True)

    with tile.TileContext(nc) as tc:
        with ExitStack() as ctx:
            sb = ctx.enter_context(tc.tile_pool(name="sb", bufs=1))
            sbw = ctx.enter_context(tc.tile_pool(name="sbw", bufs=1))
            sbg = ctx.enter_context(tc.tile_pool(name="sbg", bufs=2))
            sbb = ctx.enter_context(tc.tile_pool(name="sbb", bufs=2))
            sbu = ctx.enter_context(tc.tile_pool(name="sbu", bufs=2))
            sbo = ctx.enter_context(tc.tile_pool(name="sbo", bufs=2))
            sbd = ctx.enter_context(tc.tile_pool(name="sbd", bufs=1, space="DRAM"))
            pp = ctx.enter_context(tc.tile_pool(name="pp", bufs=1, space="PSUM"))
            ps = ctx.enter_context(tc.tile_pool(name="ps", bufs=2, space="PSUM"))
            po2 = ctx.enter_context(tc.tile_pool(name="po2", bufs=2, space="PSUM"))

            # ---- constants ----
            woff = sb.tile([CIN, K2, 18], BF16)
            nc.scalar.dma_start(out=woff[:, :, :], in_=woff_ext[:, :, :])
            wr = sb.tile([CIN, K2, 2, 128], BF16)
            nc.scalar.dma_start(out=wr[:, :, :, :], in_=wr_ext[:, :, :, :])
            idb = sb.tile([128, 128], BF16)
            nc.scalar.dma_start(out=idb[:, :], in_=idb_ext[:, :])
            ybk = sb.tile([128, 288], F32)
            nc.scalar.dma_start(out=ybk[:, :], in_=ybk_ext[:, :])
            xbk = sb.tile([128, 288], F32)
            nc.scalar.dma_start(out=xbk[:, :], in_=xbk_ext[:, :])

            # ---- x -> padded bf16 image + dual-phase pair source ----
            xf = sb.tile([CIN, HW], F32)
            nc.scalar.dma_start(out=xf[:, :], in_=x_ext[:, :])
            src = sb.tile([CIN, SRCLEN], BF16)
            nc.vector.memset(src[:, :], 0.0)
            nc.vector.tensor_copy(
                src[:, :FLAT].rearrange("c (r q) -> c r q", r=PADW)[:, 3:67, 3:67],
                xf[:, :].rearrange("c (r q) -> c r q", r=H),
            )
            nc.vector.tensor_copy(src[:, FLAT : FLAT + FLAT - 1], src[:, 1:FLAT])
            src32 = src[:, :].bitcast(F32)            # [128, 4904] f32 (bf16 pairs)

            img = src[:, :FLAT].rearrange("c (r q) -> c r q", r=PADW)

            # ---- offset conv -> off [18, 4096] bf16 ----
            off = sb.tile([18, HW], BF16)
            for t8 in range(8):
                y0 = t8 * 8
                pof = pp.tile([18, 512], F32, tag="poff")
                for t in range(K2):
                    ky, kx = t // 3 - 1, t % 3 - 1
                    rhs = img[:, y0 + ky + 3 : y0 + ky + 11, kx + 3 : kx + 67]
                    nc.tensor.matmul(pof[:, :], woff[:, t, :], rhs,
                                     start=(t == 0), stop=(t == K2 - 1))
                nc.scalar.copy(off[:, t8 * 512 : (t8 + 1) * 512], pof[:, :])

            # ---- transpose offsets -> offT [128, (pt, t, 2)] f32 ----
            offT = sb.tile([128, NPT, K2, 2], F32)
            for pt in range(NPT):
                pot = pp.tile([128, 18], BF16, tag="potr")
                nc.tensor.transpose(out=pot[:, :], in_=off[:, pt * 128 : (pt + 1) * 128],
                                    identity=idb[0:18, 0:18])
                nc.scalar.copy(offT[:, pt, :, :], pot[:, :])

            # ---- coords / weights / indices ([128, 288] f32) ----
            def cwt(name):
                return sbw.tile([128, 288], F32, tag=name, name=name)

            oy = offT[:, :, :, 0].rearrange("p a b -> p (a b)")   # (pt, t)
            ox = offT[:, :, :, 1].rearrange("p a b -> p (a b)")
            py = cwt("py")
            nc.vector.tensor_tensor(py[:, :], oy, ybk[:, :], op=ALU.add)
            px = cwt("px")
            nc.vector.tensor_tensor(px[:, :], ox, xbk[:, :], op=ALU.add)
            nc.vector.tensor_scalar(py[:, :], py[:, :], -2.99, 65.99, op0=ALU.max, op1=ALU.min)
            nc.vector.tensor_scalar(px[:, :], px[:, :], -2.99, 65.99, op0=ALU.max, op1=ALU.min)
            it = sbw.tile([128, 288], I32, tag="it", name="it")
            r0 = cwt("r0")
            nc.vector.tensor_scalar(it[:, :], py[:, :], -0.5, None, op0=ALU.add)
            nc.vector.tensor_copy(r0[:, :], it[:, :])
            wy = cwt("wy")
            nc.vector.tensor_tensor(wy[:, :], py[:, :], r0[:, :], op=ALU.subtract)
            c0 = cwt("c0")
            nc.vector.tensor_scalar(it[:, :], px[:, :], -0.5, None, op0=ALU.add)
            nc.vector.tensor_copy(c0[:, :], it[:, :])
            wx = cwt("wx")
            nc.vector.tensor_tensor(wx[:, :], px[:, :], c0[:, :], op=ALU.subtract)
            # flat+213 -> dual-phase pair index pf0 = (flat%2)*4900/2*2... (see baseline)
            t1 = cwt("t1")
            nc.vector.scalar_tensor_tensor(t1[:, :], r0[:, :], 70.0, c0[:, :], ALU.mult, ALU.add)
            t2 = cwt("t2")
            nc.vector.tensor_scalar(t2[:, :], t1[:, :], 213.0, 0.5, op0=ALU.add, op1=ALU.mult)
            hh = cwt("hh")
            nc.vector.tensor_scalar(it[:, :], t2[:, :], -0.25, None, op0=ALU.add)
            nc.vector.tensor_copy(hh[:, :], it[:, :])
            m2 = cwt("t1")   # reuse t1 storage (t1 dead)
            nc.vector.tensor_tensor(m2[:, :], t2[:, :], hh[:, :], op=ALU.subtract)
            pf0 = cwt("t2")  # reuse t2 storage (t2 dead)
            nc.vector.scalar_tensor_tensor(pf0[:, :], m2[:, :], 4900.0, hh[:, :], ALU.mult, ALU.add)
            # pf8 [128, pt, row, k] f32: top-row and bottom-row pair indices
            pf8 = sbw.tile([128, NPT, 2, K2], F32, tag="pf8", name="pf8")
            pf0v2 = pf0[:, :].rearrange("p (t k) -> p t k", t=NPT)
            nc.vector.tensor_copy(pf8[:, :, 0, :], pf0v2)
            nc.vector.tensor_scalar(pf8[:, :, 1, :], pf0v2, 35.0, None, op0=ALU.add)

            # ---- corner weight planes Wev/Wod [128, (pt,t), row] bf16 ----
            Wev = sbw.tile([128, 288, 2], BF16, tag="Wev", name="Wev")
            Wod = sbw.tile([128, 288, 2], BF16, tag="Wod", name="Wod")
            w11 = cwt("w11")
            nc.vector.tensor_tensor(w11[:, :], wy[:, :], wx[:, :], op=ALU.mult)
            nc.vector.tensor_copy(Wod[:, :, 1], w11[:, :])
            nc.vector.tensor_tensor(Wod[:, :, 0], wx[:, :], w11[:, :], op=ALU.subtract)
            nc.vector.tensor_tensor(Wev[:, :, 1], wy[:, :], w11[:, :], op=ALU.subtract)
            s4 = cwt("s4")
            nc.vector.tensor_tensor(s4[:, :], wy[:, :], wx[:, :], op=ALU.add)
            nc.vector.tensor_tensor(s4[:, :], w11[:, :], s4[:, :], op=ALU.subtract)
            nc.vector.tensor_scalar(Wev[:, :, 0], s4[:, :], 1.0, None, op0=ALU.add)

            # ---- transpose weight planes, interleave even/odd ----
            # wstage[q', ch, pix, eo], q = (pt*9+t)*2+row = 126*ch + q'
            wstage = sb.tile([128, 5, 128, 2], BF16)
            WevF = Wev[:, :, :].rearrange("p a b -> p (a b)")    # [128, 576]
            WodF = Wod[:, :, :].rearrange("p a b -> p (a b)")
            for ch in range(5):
                lo = ch * 126
                n = min(576, lo + 126) - lo
                pse = ps.tile([128, 128], BF16, tag="wtp")
                nc.tensor.transpose(out=pse[0:n, :], in_=WevF[:, lo : lo + n],
                                    identity=idb[:, :])
                nc.scalar.copy(wstage[0:n, ch, :, 0], pse[0:n, :])
                pso_ = ps.tile([128, 128], BF16, tag="wtp")
                nc.tensor.transpose(out=pso_[0:n, :], in_=WodF[:, lo : lo + n],
                                    identity=idb[:, :])
                nc.scalar.copy(wstage[0:n, ch, :, 1], pso_[0:n, :])

            # ---- collapse per pixel tile to DRAM: wsc [32, t, row, pix, eo] ----
            # (DRAM scratch tile so the per-tile partition-broadcast read is a
            # stride-0 DRAM AP and Tile tracks the write->read dependency)
            wsc_ext = sbd.tile([32, K2, 2, 128, 2], BF16)
            for pt in range(NPT):
                ch = pt // 7
                q0 = (pt - 7 * ch) * 18
                nc.sync.dma_start(
                    out=wsc_ext[pt, :, :, :, :],
                    in_=wstage[q0 : q0 + 18, ch, :, :],
                )

            # ---- wrapped gather indices WI [128, row, pt, k, s] i16 ----
            # partition fold (pixel p = s*16+q -> partition q, free s) via 8
            # partition-shift DMAs into an s-major staging tile, then one
            # strided DVE copy to put s innermost (and cast f32 -> i16).
            stg8 = sbw.tile([16, 8, NPT, 2, K2], F32, tag="stg8", name="stg8")
            for s in range(8):
                nc.sync.dma_start(
                    out=stg8[:, s, :, :, :],
                    in_=pf8[s * 16 : s * 16 + 16, :, :, :],
                )
            WI = sb.tile([128, NPT, 2, K2, 8], I16)
            nc.vector.tensor_copy(
                WI[0:16, :, :, :, :],
                stg8[:, :, :, :, :].rearrange("q s b r k -> q b r k s"),
            )
            for rep in [16, 32, 64]:
                nc.scalar.dma_start(out=WI[rep : 2 * rep], in_=WI[0:rep])

            # ---- main loop over pixel tiles ----
            NIDX = 2 * K2 * 128      # 2304 idx per pt (both rows in one call)
            STAGE = int(os.environ.get("DEFORM_STAGE", "3"))
            # tiles whose vertical pair-sum rides the PE (4-phase matmuls)
            # instead of a DVE add — balances the DVE against the PE
            PESPLIT = int(os.environ.get("DEFORM_PESPLIT", "12"))
            for pt in range(NPT if STAGE >= 2 else 0):
                gath = sbg.tile([128, 2, K2, 128], F32, tag="gath", name="gath")
                nc.gpsimd.ap_gather(
                    gath[:, :, :, :].rearrange("c r k (p one) -> c (r k p) one", one=1),
                    src32[:, :].rearrange("c (e one) -> c e one", one=1),
                    WI[:, pt, :, :, :].rearrange("c r k s -> c (r k s)"),
                    channels=128, num_elems=4904, d=1, num_idxs=NIDX,
                )
                if STAGE == 2:
                    if pt == 0:
                        nc.gpsimd.dma_start(
                            out=out_ext[0, :, 0:2048],
                            in_=gath[:, :, :, :].rearrange("c r k p -> c (r k p)")[:, 0:2048])
                    continue
                wb = sbb.tile([128, K2, 2, 128, 2], BF16, tag="wb", name="wb")
                nc.sync.dma_start(
                    out=wb[:, :, :, :, :],
                    in_=wsc_ext[pt, :, :, :, :].partition_broadcast(128),
                )
                if STAGE == 25:
                    if pt == 0:
                        nc.gpsimd.dma_start(
                            out=out_ext[0, :, 0:2304],
                            in_=wb[:, :, :, :, :].rearrange("c a b p e -> c (a b p e)").bitcast(F32))
                    continue
                qp = sbu.tile([128, 2, K2, 256], BF16, tag="qp", name="qp")
                for row in range(2):
                    nc.vector.tensor_tensor(
                        qp[:, row, :, :],
                        gath[:, row, :, :].bitcast(BF16),
                        wb[:, :, row, :, :].rearrange("c k p e -> c k (p e)"),
                        op=ALU.mult)
                outp = {}
                for hf in range(2):
                    outp[hf] = po2.tile([128, 128], F32, tag=f"out{hf}", name=f"outp{hf}")
                if PESPLIT > 0 and (pt * PESPLIT) // NPT != ((pt + 1) * PESPLIT) // NPT:
                    # PE does the vertical pair-sum: 4 accumulating phases
                    qv = qp[:, :, :, :].rearrange("c r k (p e) -> c r k p e", e=2)
                    for t in range(K2):
                        for hf in range(2):
                            for row in range(2):
                                for ph in range(2):
                                    nc.tensor.matmul(
                                        outp[hf][:, :], wr[:, t, hf, :],
                                        qv[:, row, t, :, ph],
                                        start=(t == 0 and row == 0 and ph == 0),
                                        stop=(t == K2 - 1 and row == 1 and ph == 1))
                else:
                    v2 = sbu.tile([128, K2, 128, 2], BF16, tag="v2", name="v2")
                    nc.vector.tensor_tensor(
                        v2[:, :, :, :],
                        qp[:, 0, :, :].rearrange("c k (p e) -> c k p e", e=2),
                        qp[:, 1, :, :].rearrange("c k (p e) -> c k p e", e=2),
                        op=ALU.add)
                    for t in range(K2):
                        for hf in range(2):
                            for ph in range(2):
                                nc.tensor.matmul(
                                    outp[hf][:, :], wr[:, t, hf, :], v2[:, t, :, ph],
                                    start=(t == 0 and ph == 0),
                                    stop=(t == K2 - 1 and ph == 1))
                lg = pt % OG
                if lg == 0:
                    stg = sbo.tile([128, 2, OG, 128], F32, tag="stg", name="stg")
                for hf in range(2):
                    nc.scalar.copy(stg[:, hf, lg, :], outp[hf][:, :])
                if lg == OG - 1:
                    g0 = (pt // OG) * OG
                    nc.sync.dma_start(
                        out=out_ext[:, :, g0 * 128 : (g0 + OG) * 128]
                        .rearrange("h o p -> o h p"),
                        in_=stg[:, :, :, :])
            if STAGE == 1:
                nc.gpsimd.dma_start(out=out_ext[0, :, 0:288], in_=pf0[:, :])
                nc.gpsimd.dma_start(out=out_ext[0, :, 288:432],
                                    in_=Wev[:, :, :].bitcast(F32)[:, 0:144])
    nc.compile()
    return nc


def _prep_consts():
    yb = (np.arange(HW) // W).reshape(NPT, 128).T
    xb = (np.arange(HW) % W).reshape(NPT, 128).T
    ky = np.arange(K2) // 3 - 1
    kx = np.arange(K2) % 3 - 1
    ybk = (yb[:, :, None] + ky[None, None, :]).reshape(128, 288).astype(np.float32)
    xbk = (xb[:, :, None] + kx[None, None, :]).reshape(128, 288).astype(np.float32)
    idb = np.eye(128, dtype=ml_dtypes.bfloat16)
    return ybk, xbk, idb


def kernel(x, offset_w, offset_b, deform_w, deform_b):
    x = np.asarray(x, dtype=np.float32)
    offset_w = np.asarray(offset_w, dtype=np.float32)
    offset_b = np.asarray(offset_b, dtype=np.float32)
    deform_w = np.asarray(deform_w, dtype=np.float32)
    deform_b = np.asarray(deform_b, dtype=np.float32)

    if "nc" not in _cache:
        _cache["nc"] = _build_program()
    nc = _cache["nc"]

    ybk, xbk, idb = _prep_consts()
    oby = offset_b.reshape(9, 2)[:, 0]
    obx = offset_b.reshape(9, 2)[:, 1]
    ybk2 = (ybk.reshape(128, 32, 9) + oby[None, None, :]).reshape(128, 288).astype(np.float32)
    xbk2 = (xbk.reshape(128, 32, 9) + obx[None, None, :]).reshape(128, 288).astype(np.float32)
    woff = offset_w.reshape(18, CIN, 3, 3).transpose(1, 2, 3, 0).reshape(CIN, K2, 18)
    woff = np.ascontiguousarray(woff).astype(ml_dtypes.bfloat16)
    wrh = deform_w.reshape(COUT, CIN, K2).transpose(1, 2, 0).reshape(CIN, K2, 2, 128)
    wrh = np.ascontiguousarray(wrh).astype(ml_dtypes.bfloat16)

    in_maps = []
    for b in range(B):
        in_maps.append({
            "x": np.ascontiguousarray(x[b].reshape(CIN, HW)),
            "woff": woff, "wr": wrh,
            "idb": idb, "ybk": ybk2, "xbk": xbk2,
        })
    res = run_bass_kernel_spmd(nc, in_maps, list(range(B)))
    out = np.stack([r["out"].reshape(COUT, H, W) for r in res.results])
    out = out + deform_b[None, :, None, None]
    return out.astype(np.float32)


# revision 10
# speedup vs baseline: 1.0904x; 1.0904x over previous
"""DeformableConv Trainium2 Bass kernel (transpose-free main loop).

B=8, Cin=128, Cout=256, H=W=64, K=3. Data-parallel over batch: core b
processes sample b. Per-core pipeline:

  1. offset conv (PE, 9 shifted-AP matmuls, M=18) -> offsets [18, 4096]
  2. PE-transpose offsets -> [128 pix, (pt,t,2)] layout; sampling
     coords, bilinear corner weights, pair-gather indices on DVE (fp32).
  3. Corner weights land in two bf16 planes (even = w00/w10 and
     odd = w01/w11, both [pix, (pt,t,row)]), are PE-transposed to
     (pt,t,row)-major, interleaved into (even,odd) pairs by strided ACT
     copies, collapsed per pixel-tile to one partition, and finally
     DMA partition-broadcast so each tile gets a [128chan, t, row,
     pix, 2] weight tile (identical rows on all channel partitions).
  4. ap_gather (gpsimd, d=1 float32 = one bf16 horizontal PAIR per
     index) fetches top-row and bottom-row corner pairs from the
     dual-phase padded image, one call per (row, pixel tile).
  5. DVE: two 2x-mode bf16 multiplies (pairs x weight-pairs) and one
     add implement the full bilinear weighting in channel-major
     layout; no per-tile transposes anywhere.
  6. PE conv: out[o,p] accumulated over 9 taps x 2 horizontal phases
     in PSUM (the phase pair-sum rides on the matmul accumulation);
     ACT drains to an 8-tile stage, DMA stores.
"""

import sys, os
sys.path.insert(0, "/opt/trn_rl_repo")

import numpy as np
import ml_dtypes

import concourse.bass as bass
import concourse.tile as tile
from concourse import bacc, mybir
from concourse.bass_utils import run_bass_kernel_spmd
from contextlib import ExitStack

F32 = mybir.dt.float32
BF16 = mybir.dt.bfloat16
I16 = mybir.dt.int16
I32 = mybir.dt.int32
ALU = mybir.AluOpType

B, CIN, COUT, H, W = 8, 128, 256, 64, 64
K2 = 9
HW = H * W                 # 4096
PADW = 70
FLAT = PADW * PADW         # 4900
SRCLEN = 9808              # [0:4900] padded img, [4900:9799] img shifted 1
NPT = 32                   # pixel tiles of 128
OG = 8                     # pixel tiles per output DMA group

_cache = {}


def _build_program():
    nc = bacc.Bacc("TRN2", target_bir_lowering=False, debug=False, num_devices=B)

    x_ext = nc.declare_dram_parameter("x", [CIN, HW], F32, isOutput=False)
    woff_ext = nc.declare_dram_parameter("woff", [CIN, K2, 18], BF16, isOutput=False)
    wr_ext = nc.declare_dram_parameter("wr", [CIN, K2, 2, 128], BF16, isOutput=False)
    idb_ext = nc.declare_dram_parameter("idb", [128, 128], BF16, isOutput=False)
    ybk_ext = nc.declare_dram_parameter("ybk", [128, 288], F32, isOutput=False)
    xbk_ext = nc.declare_dram_parameter("xbk", [128, 288], F32, isOutput=False)
    out_ext = nc.declare_dram_parameter("out", [2, 128, HW], F32, isOutput=True)

    with tile.TileContext(nc) as tc:
        with ExitStack() as ctx:
            sb = ctx.enter_context(tc.tile_pool(name="sb", bufs=1))
            sbw = ctx.enter_context(tc.tile_pool(name="sbw", bufs=1))
            sbg = ctx.enter_context(tc.tile_pool(name="sbg", bufs=2))
            sbb = ctx.enter_context(tc.tile_pool(name="sbb", bufs=3))
            sbu = ctx.enter_context(tc.tile_pool(name="sbu", bufs=3))
            sbo = ctx.enter_context(tc.tile_pool(name="sbo", bufs=2))
            sbd = ctx.enter_context(tc.tile_pool(name="sbd", bufs=1, space="DRAM"))
            pp = ctx.enter_context(tc.tile_pool(name="pp", bufs=1, space="PSUM"))
            ps = ctx.enter_context(tc.tile_pool(name="ps", bufs=2, space="PSUM"))
            po2 = ctx.enter_context(tc.tile_pool(name="po2", bufs=2, space="PSUM"))

            # ---- constants ----
            woff = sb.tile([CIN, K2, 18], BF16)
            nc.scalar.dma_start(out=woff[:, :, :], in_=woff_ext[:, :, :])
            wr = sb.tile([CIN, K2, 2, 128], BF16)
            nc.scalar.dma_start(out=wr[:, :, :, :], in_=wr_ext[:, :, :, :])
            idb = sb.tile([128, 128], BF16)
            nc.scalar.dma_start(out=idb[:, :], in_=idb_ext[:, :])
            ybk = sb.tile([128, 288], F32)
            nc.scalar.dma_start(out=ybk[:, :], in_=ybk_ext[:, :])
            xbk = sb.tile([128, 288], F32)
            nc.scalar.dma_start(out=xbk[:, :], in_=xbk_ext[:, :])

            # ---- x -> padded bf16 image + dual-phase pair source ----
            xf = sb.tile([CIN, HW], F32)
            nc.scalar.dma_start(out=xf[:, :], in_=x_ext[:, :])
            src = sb.tile([CIN, SRCLEN], BF16)
            nc.vector.memset(src[:, :], 0.0)
            nc.vector.tensor_copy(
                src[:, :FLAT].rearrange("c (r q) -> c r q", r=PADW)[:, 3:67, 3:67],
                xf[:, :].rearrange("c (r q) -> c r q", r=H),
            )
            nc.vector.tensor_copy(src[:, FLAT : FLAT + FLAT - 1], src[:, 1:FLAT])
            src32 = src[:, :].bitcast(F32)            # [128, 4904] f32 (bf16 pairs)

            img = src[:, :FLAT].rearrange("c (r q) -> c r q", r=PADW)

            # ---- offset conv -> off [18, 4096] bf16 ----
            off = sb.tile([18, HW], BF16)
            for t8 in range(8):
                y0 = t8 * 8
                pof = pp.tile([18, 512], F32, tag="poff")
                for t in range(K2):
                    ky, kx = t // 3 - 1, t % 3 - 1
                    rhs = img[:, y0 + ky + 3 : y0 + ky + 11, kx + 3 : kx + 67]
                    nc.tensor.matmul(pof[:, :], woff[:, t, :], rhs,
                                     start=(t == 0), stop=(t == K2 - 1))
                nc.scalar.copy(off[:, t8 * 512 : (t8 + 1) * 512], pof[:, :])

            # ---- transpose offsets -> offT [128, (pt, t, 2)] f32 ----
            offT = sb.tile([128, NPT, K2, 2], F32)
            for pt in range(NPT):
                pot = pp.tile([128, 18], BF16, tag="potr")
                nc.tensor.transpose(out=pot[:, :], in_=off[:, pt * 128 : (pt + 1) * 128],
                                    identity=idb[0:18, 0:18])
                nc.scalar.copy(offT[:, pt, :, :], pot[:, :])

            # ---- coords / weights / indices ([128, 288] f32) ----
            def cwt(name):
                return sbw.tile([128, 288], F32, tag=name, name=name)

            oy = offT[:, :, :, 0].rearrange("p a b -> p (a b)")   # (pt, t)
            ox = offT[:, :, :, 1].rearrange("p a b -> p (a b)")
            py = cwt("py")
            nc.vector.tensor_tensor(py[:, :], oy, ybk[:, :], op=ALU.add)
            px = cwt("px")
            nc.vector.tensor_tensor(px[:, :], ox, xbk[:, :], op=ALU.add)
            nc.vector.tensor_scalar(py[:, :], py[:, :], -2.99, 65.99, op0=ALU.max, op1=ALU.min)
            nc.vector.tensor_scalar(px[:, :], px[:, :], -2.99, 65.99, op0=ALU.max, op1=ALU.min)
            it = sbw.tile([128, 288], I32, tag="it", name="it")
            r0 = cwt("r0")
            nc.vector.tensor_scalar(it[:, :], py[:, :], -0.5, None, op0=ALU.add)
            nc.vector.tensor_copy(r0[:, :], it[:, :])
            wy = cwt("wy")
            nc.vector.tensor_tensor(wy[:, :], py[:, :], r0[:, :], op=ALU.subtract)
            c0 = cwt("c0")
            nc.vector.tensor_scalar(it[:, :], px[:, :], -0.5, None, op0=ALU.add)
            nc.vector.tensor_copy(c0[:, :], it[:, :])
            wx = cwt("wx")
            nc.vector.tensor_tensor(wx[:, :], px[:, :], c0[:, :], op=ALU.subtract)
            # flat+213 -> dual-phase pair index pf0 = (flat%2)*4900/2*2... (see baseline)
            t1 = cwt("t1")
            nc.vector.scalar_tensor_tensor(t1[:, :], r0[:, :], 70.0, c0[:, :], ALU.mult, ALU.add)
            t2 = cwt("t2")
            nc.vector.tensor_scalar(t2[:, :], t1[:, :], 213.0, 0.5, op0=ALU.add, op1=ALU.mult)
            hh = cwt("hh")
            nc.vector.tensor_scalar(it[:, :], t2[:, :], -0.25, None, op0=ALU.add)
            nc.vector.tensor_copy(hh[:, :], it[:, :])
            m2 = cwt("t1")   # reuse t1 storage (t1 dead)
            nc.vector.tensor_tensor(m2[:, :], t2[:, :], hh[:, :], op=ALU.subtract)
            pf0 = cwt("t2")  # reuse t2 storage (t2 dead)
            nc.vector.scalar_tensor_tensor(pf0[:, :], m2[:, :], 4900.0, hh[:, :], ALU.mult, ALU.add)
            # pf8 [128, pt, row, k] f32: top-row and bottom-row pair indices
            pf8 = sbw.tile([128, NPT, 2, K2], F32, tag="pf8", name="pf8")
            pf0v2 = pf0[:, :].rearrange("p (t k) -> p t k", t=NPT)
            nc.vector.tensor_copy(pf8[:, :, 0, :], pf0v2)
            nc.vector.tensor_scalar(pf8[:, :, 1, :], pf0v2, 35.0, None, op0=ALU.add)

            # ---- corner weight planes Wev/Wod [128, (pt,t), row] bf16 ----
            Wev = sbw.tile([128, 288, 2], BF16, tag="Wev", name="Wev")
            Wod = sbw.tile([128, 288, 2], BF16, tag="Wod", name="Wod")
            w11 = cwt("w11")
            nc.vector.tensor_tensor(w11[:, :], wy[:, :], wx[:, :], op=ALU.mult)
            nc.vector.tensor_copy(Wod[:, :, 1], w11[:, :])
            nc.vector.tensor_tensor(Wod[:, :, 0], wx[:, :], w11[:, :], op=ALU.subtract)
            nc.vector.tensor_tensor(Wev[:, :, 1], wy[:, :], w11[:, :], op=ALU.subtract)
            s4 = cwt("s4")
            nc.vector.tensor_tensor(s4[:, :], wy[:, :], wx[:, :], op=ALU.add)
            nc.vector.tensor_tensor(s4[:, :], w11[:, :], s4[:, :], op=ALU.subtract)
            nc.vector.tensor_scalar(Wev[:, :, 0], s4[:, :], 1.0, None, op0=ALU.add)

            # ---- transpose weight planes, interleave even/odd ----
            # wstage[q', ch, pix, eo], q = (pt*9+t)*2+row = 126*ch + q'
            wstage = sb.tile([128, 5, 128, 2], BF16)
            WevF = Wev[:, :, :].rearrange("p a b -> p (a b)")    # [128, 576]
            WodF = Wod[:, :, :].rearrange("p a b -> p (a b)")
            for ch in range(5):
                lo = ch * 126
                n = min(576, lo + 126) - lo
                pse = ps.tile([128, 128], BF16, tag="wtp")
                nc.tensor.transpose(out=pse[0:n, :], in_=WevF[:, lo : lo + n],
                                    identity=idb[:, :])
                nc.scalar.copy(wstage[0:n, ch, :, 0], pse[0:n, :])
                pso_ = ps.tile([128, 128], BF16, tag="wtp")
                nc.tensor.transpose(out=pso_[0:n, :], in_=WodF[:, lo : lo + n],
                                    identity=idb[:, :])
                nc.scalar.copy(wstage[0:n, ch, :, 1], pso_[0:n, :])

            # ---- collapse per pixel tile to DRAM: wsc [32, t, row, pix, eo] ----
            # (DRAM scratch tile so the per-tile partition-broadcast read is a
            # stride-0 DRAM AP and Tile tracks the write->read dependency)
            wsc_ext = sbd.tile([32, K2, 2, 128, 2], BF16)
            for pt in range(NPT):
                ch = pt // 7
                q0 = (pt - 7 * ch) * 18
                nc.sync.dma_start(
                    out=wsc_ext[pt, :, :, :, :],
                    in_=wstage[q0 : q0 + 18, ch, :, :],
                )

            # ---- wrapped gather indices WI [128, row, pt, k, s] i16 ----
            # partition fold (pixel p = s*16+q -> partition q, free s) via 8
            # partition-shift DMAs into an s-major staging tile, then one
            # strided DVE copy to put s innermost (and cast f32 -> i16).
            stg8 = sbw.tile([16, 8, NPT, 2, K2], F32, tag="stg8", name="stg8")
            for s in range(8):
                nc.sync.dma_start(
                    out=stg8[:, s, :, :, :],
                    in_=pf8[s * 16 : s * 16 + 16, :, :, :],
                )
            WI = sb.tile([128, NPT, 2, K2, 8], I16)
            nc.vector.tensor_copy(
                WI[0:16, :, :, :, :],
                stg8[:, :, :, :, :].rearrange("q s b r k -> q b r k s"),
            )
            for rep in [16, 32, 64]:
                nc.scalar.dma_start(out=WI[rep : 2 * rep], in_=WI[0:rep])

            # ---- main loop over pixel tiles ----
            NIDX = 2 * K2 * 128      # 2304 idx per pt (both rows in one call)
            STAGE = int(os.environ.get("DEFORM_STAGE", "3"))
            # tiles whose vertical pair-sum rides the PE (4-phase matmuls)
            # instead of a DVE add — balances the DVE against the PE
            PESPLIT = int(os.environ.get("DEFORM_PESPLIT", "12"))
            for pt in range(NPT if STAGE >= 2 else 0):
                gath = sbg.tile([128, 2, K2, 128], F32, tag="gath", name="gath")
                nc.gpsimd.ap_gather(
                    gath[:, :, :, :].rearrange("c r k (p one) -> c (r k p) one", one=1),
                    src32[:, :].rearrange("c (e one) -> c e one", one=1),
                    WI[:, pt, :, :, :].rearrange("c r k s -> c (r k s)"),
                    channels=128, num_elems=4904, d=1, num_idxs=NIDX,
                )
                if STAGE == 2:
                    if pt == 0:
                        nc.gpsimd.dma_start(
                            out=out_ext[0, :, 0:2048],
                            in_=gath[:, :, :, :].rearrange("c r k p -> c (r k p)")[:, 0:2048])
                    continue
                wb = sbb.tile([128, K2, 2, 128, 2], BF16, tag="wb", name="wb")
                nc.sync.dma_start(
                    out=wb[:, :, :, :, :],
                    in_=wsc_ext[pt, :, :, :, :].partition_broadcast(128),
                )
                if STAGE == 25:
                    if pt == 0:
                        nc.gpsimd.dma_start(
                            out=out_ext[0, :, 0:2304],
                            in_=wb[:, :, :, :, :].rearrange("c a b p e -> c (a b p e)").bitcast(F32))
                    continue
                qp = sbu.tile([128, 2, K2, 256], BF16, tag="qp", name="qp")
                for row in range(2):
                    nc.vector.tensor_tensor(
                        qp[:, row, :, :],
                        gath[:, row, :, :].bitcast(BF16),
                        wb[:, :, row, :, :].rearrange("c k p e -> c k (p e)"),
                        op=ALU.mult)
                outp = {}
                for hf in range(2):
                    outp[hf] = po2.tile([128, 128], F32, tag=f"out{hf}", name=f"outp{hf}")
                if PESPLIT > 0 and (pt * PESPLIT) // NPT != ((pt + 1) * PESPLIT) // NPT:
                    # PE does the vertical pair-sum: 4 accumulating phases
                    qv = qp[:, :, :, :].rearrange("c r k (p e) -> c r k p e", e=2)
                    for t in range(K2):
                        for hf in range(2):
                            for row in range(2):
                                for ph in range(2):
                                    nc.tensor.matmul(
                                        outp[hf][:, :], wr[:, t, hf, :],
                                        qv[:, row, t, :, ph],
                                        start=(t == 0 and row == 0 and ph == 0),
                                        stop=(t == K2 - 1 and row == 1 and ph == 1))
                else:
                    v2 = sbu.tile([128, K2, 128, 2], BF16, tag="v2", name="v2")
                    nc.vector.tensor_tensor(
                        v2[:, :, :, :],
                        qp[:, 0, :, :].rearrange("c k (p e) -> c k p e", e=2),
                        qp[:, 1, :, :].rearrange("c k (p e) -> c k p e", e=2),
                        op=ALU.add)
                    for t in range(K2):
                        for hf in range(2):
                            for ph in range(2):
                                nc.tensor.matmul(
                                    outp[hf][:, :], wr[:, t, hf, :], v2[:, t, :, ph],
                                    start=(t == 0 and ph == 0),
                                    stop=(t == K2 - 1 and ph == 1))
                lg = pt % OG
                if lg == 0:
                    stg = sbo.tile([128, 2, OG, 128], F32, tag="stg", name="stg")
                for hf in range(2):
                    nc.scalar.copy(stg[:, hf, lg, :], outp[hf][:, :])
                if lg == OG - 1:
                    g0 = (pt // OG) * OG
                    nc.scalar.dma_start(
                        out=out_ext[:, :, g0 * 128 : (g0 + OG) * 128]
                        .rearrange("h o p -> o h p"),
                        in_=stg[:, :, :, :])
            if STAGE == 1:
                nc.gpsimd.dma_start(out=out_ext[0, :, 0:288], in_=pf0[:, :])
                nc.gpsimd.dma_start(out=out_ext[0, :, 288:432],
                                    in_=Wev[:, :, :].bitcast(F32)[:, 0:144])
    nc.compile()
    return nc


def _prep_consts():
    yb = (np.arange(HW) // W).reshape(NPT, 128).T
    xb = (np.arange(HW) % W).reshape(NPT, 128).T
    ky = np.arange(K2) // 3 - 1
    kx = np.arange(K2) % 3 - 1
    ybk = (yb[:, :, None] + ky[None, None, :]).reshape(128, 288).astype(np.float32)
    xbk = (xb[:, :, None] + kx[None, None, :]).reshape(128, 288).astype(np.float32)
    idb = np.eye(128, dtype=ml_dtypes.bfloat16)
    return ybk, xbk, idb


def kernel(x, offset_w, offset_b, deform_w, deform_b):
    x = np.asarray(x, dtype=np.float32)
    offset_w = np.asarray(offset_w, dtype=np.float32)
    offset_b = np.asarray(offset_b, dtype=np.float32)
    deform_w = np.asarray(deform_w, dtype=np.float32)
    deform_b = np.asarray(deform_b, dtype=np.float32)

    if "nc" not in _cache:
        _cache["nc"] = _build_program()
    nc = _cache["nc"]

    ybk, xbk, idb = _prep_consts()
    oby = offset_b.reshape(9, 2)[:, 0]
    obx = offset_b.reshape(9, 2)[:, 1]
    ybk2 = (ybk.reshape(128, 32, 9) + oby[None, None, :]).reshape(128, 288).astype(np.float32)
    xbk2 = (xbk.reshape(128, 32, 9) + obx[None, None, :]).reshape(128, 288).astype(np.float32)
    woff = offset_w.reshape(18, CIN, 3, 3).transpose(1, 2, 3, 0).reshape(CIN, K2, 18)
    woff = np.ascontiguousarray(woff).astype(ml_dtypes.bfloat16)
    wrh = deform_w.reshape(COUT, CIN, K2).transpose(1, 2, 0).reshape(CIN, K2, 2, 128)
    wrh = np.ascontiguousarray(wrh).astype(ml_dtypes.bfloat16)

    in_maps = []
    for b in range(B):
        in_maps.append({
            "x": np.ascontiguousarray(x[b].reshape(CIN, HW)),
            "woff": woff, "wr": wrh,
            "idb": idb, "ybk": ybk2, "xbk": xbk2,
        })
    res = run_bass_kernel_spmd(nc, in_maps, list(range(B)))
    out = np.stack([r["out"].reshape(COUT, H, W) for r in res.results])
    out = out + deform_b[None, :, None, None]
    return out.astype(np.float32)
